# revision 9
# baseline (speedup 1.0000x reference)
"""Trainium2 Bass kernel for nn_AGITransformerLayer (B=4, S=1024, H=1024, NH=16).

Distribution over 8 NeuronCores: data-parallel over the 4 batches x 2-way
tensor-parallel within each adjacent core pair (cores 2b, 2b+1 handle batch b).
Within a pair, core h (=0,1) owns:
  - main attention heads h*8..h*8+8  (16 heads of dim 64, split 8/8)
  - causal-MHA heads h*2..h*2+2      (4 heads of dim 256, split 2/2)
  - meta-MHA heads h*2..h*2+2        (4 heads of dim 256, split 2/2)
  - contraction rows h*512..h*512+512 of the causal/meta out-projections and
    of the final Wo projection.

Everything on chip lives in "transposed" layout [feature, token] so every
matmul uses natural (un-transposed) operands.  Cross-core exchange per pair
(all chunked 2-way and pipelined against compute):
  ReduceScatter(causal out-proj partials) -> + local main-attn ctx half
  meta in-projections run on the LOCAL blended half only (the host hands each
  core the matching 512 rows of mc_Win) and the q/k/v partials are summed
  with an AllReduce -- so the last AllGather is off the critical path and is
  consumed only by the final residual blend.
  AllGather(ctx half) -> full blended ctx for the final blend.
The pair's two final-output partials are summed on the host.

Engine balance: Scalar(ACT) does softmax EXPs only (it is the 2nd roofline at
~100us); all PSUM evacuations run on DVE (or Scalar when it is idle: V-proj,
meta in-proj, tail).  The softmax den->recip->broadcast roundtrip is emitted
deferred (main) / under the PV chain (causal+meta) so the PE never waits on
it.  Q/K projections use full 128-wide stationaries.

Compute dtype bf16 (fp32 PSUM accumulation); softmax denominators via an
appended ones-column on V (main heads) / explicit ones-matmuls (256-dim
heads); normalizers broadcast via indicator-matrix matmuls.
"""

import sys
import types

if "/opt/trn_rl_repo" not in sys.path:
    sys.path.insert(0, "/opt/trn_rl_repo")

import numpy as np
import ml_dtypes

import concourse.bass as bass
import concourse.tile as tile
from concourse import bacc, mybir
from concourse import bass_utils

BF16 = ml_dtypes.bfloat16
P = 128
S = 1024          # sequence length
H = 1024          # hidden dim
NH_LOC = 8        # main heads per core
HD = 64           # main head dim
CH_LOC = 2        # causal/meta heads per core
CHD = 256         # causal/meta head dim
HT = H // P       # hidden tiles (8)
TT = S // P       # token tiles (8)
QB = 2            # query blocks of 512
QW = 512          # query block width
KT = S // P       # key tiles (8)
N_CORES = 8

DT = mybir.dt.bfloat16
F32 = mybir.dt.float32

# bias-pack offsets (bf16 row [1, NBIAS]); meta q/k/v slots are 1024 wide
# (biases for BOTH cores' heads -- they enter once via core 0's partials)
BQ_OFF, BK_OFF, BV_OFF = 0, 512, 1024
CQ_OFF, CK_OFF, CV_OFF = 1536, 2048, 2560
CBO_OFF = 3072
MQ_OFF, MK_OFF, MV_OFF = 4096, 5120, 6144
MBO_OFF = 7168
NBIAS = 8192

# out-proj row-tile emission order: chunk 0 = global rows 0:256 & 512:768
PT_ORDER = [0, 1, 4, 5, 2, 3, 6, 7]
# ctxT chunk consumption order = AllGather arrival order (pair 3 last)
HT_ORDER = [0, 4, 1, 5, 2, 6, 3, 7]


def _install_ntff_hook():
    """Make trace=True work under axon (inject missing antenv.axon_hooks)."""
    if "antenv.axon_hooks" in sys.modules:
        return
    try:
        mod = types.ModuleType("antenv.axon_hooks")
        mod._hook = None
        mod.set_axon_ntff_profile_hook = lambda h: setattr(mod, "_hook", h)
        mod.get_axon_ntff_profile_hook = lambda: mod._hook
        import antenv
        antenv.axon_hooks = mod
        sys.modules["antenv.axon_hooks"] = mod
        from trn_agent_boot.trn_boot import _ntff_profile_via_ctypes
        mod.set_axon_ntff_profile_hook(
            _ntff_profile_via_ctypes("/opt/axon/libaxon_pjrt.so"))
        bass_utils.upload_artifacts = lambda tmpdir: tmpdir
    except Exception:
        pass



def _emit(nc, tc, bias_on):
    """Emit the whole per-core program.  bias_on: dict of bools (graph-uniform)."""
    xt_d = nc.dram_tensor("xt", [H, S], DT, kind="ExternalInput")
    wq_d = nc.dram_tensor("wq", [H, 512], DT, kind="ExternalInput")
    wk_d = nc.dram_tensor("wk", [H, 512], DT, kind="ExternalInput")
    wv_d = nc.dram_tensor("wv", [H, 512], DT, kind="ExternalInput")
    cgw_d = nc.dram_tensor("cgw", [H, 8], DT, kind="ExternalInput")
    modb_d = nc.dram_tensor("modb", [8, 1], F32, kind="ExternalInput")
    cwq_d = nc.dram_tensor("cwq", [H, 512], DT, kind="ExternalInput")
    cwk_d = nc.dram_tensor("cwk", [H, 512], DT, kind="ExternalInput")
    cwv_d = nc.dram_tensor("cwv", [H, 512], DT, kind="ExternalInput")
    cwo_d = nc.dram_tensor("cwo", [512, H], DT, kind="ExternalInput")
    mwq_d = nc.dram_tensor("mwq", [H, 512], DT, kind="ExternalInput")
    mwk_d = nc.dram_tensor("mwk", [H, 512], DT, kind="ExternalInput")
    mwv_d = nc.dram_tensor("mwv", [H, 512], DT, kind="ExternalInput")
    mwo_d = nc.dram_tensor("mwo", [512, H], DT, kind="ExternalInput")
    wo_d = nc.dram_tensor("wo", [H, H], DT, kind="ExternalInput")
    biasp_d = nc.dram_tensor("biasp", [1, NBIAS], DT, kind="ExternalInput")
    out_d = nc.dram_tensor("out", [S, H], DT, kind="ExternalOutput")

    def r3(d):  # [R, C] dram -> [P, R//P, C] view
        return d.ap().rearrange("(o p) c -> p o c", p=P)

    out_v = out_d.ap().rearrange("(o p) c -> p o c", p=P)

    mult, add = mybir.AluOpType.mult, mybir.AluOpType.add
    EXP = mybir.ActivationFunctionType.Exp
    SIG = mybir.ActivationFunctionType.Sigmoid

    import contextlib
    stack = contextlib.ExitStack()
    with stack:
        cpool = stack.enter_context(tc.tile_pool(name="const", bufs=1))
        ones_row = cpool.tile([1, 512], DT)
        nc.vector.memset(ones_row[:], 1.0)
        ones_col = cpool.tile([P, 1], DT)
        nc.vector.memset(ones_col[:], 1.0)
        # indicator: row 0 -> out partitions 0:64, row 1 -> 64:128
        # (built on partition 0 and reshaped into 2 partitions via DMA --
        # engines may not address a single partition above 0 directly)
        indic_st = cpool.tile([1, 2 * P], DT)
        nc.vector.memset(indic_st[:], 0.0)
        nc.vector.memset(indic_st[0:1, 0:64], 1.0)
        nc.vector.memset(indic_st[0:1, P + 64:2 * P], 1.0)
        indic = cpool.tile([2, P], DT)
        nc.sync.dma_start(indic[:], indic_st[0:1, :])
        modb_sb = cpool.tile([8, 1], F32)

        # persistent across stages
        apool = stack.enter_context(tc.tile_pool(name="persist", bufs=1))
        xt_sb = apool.tile([P, HT, S], DT, tag="xt_mp")
        ch_sb = apool.tile([P, 4, S], DT)      # blended ctx^T, my half rows
        ctxT_sb = apool.tile([P, HT, S], DT)

        # all weights, prefetched; causal->meta->wo share slots via tags
        wpool = stack.enter_context(tc.tile_pool(name="wall", bufs=1))
        wq_sb = wpool.tile([P, HT, 512], DT)
        wk_sb = wpool.tile([P, HT, 512], DT)
        wv_sb = wpool.tile([P, HT, 512], DT)
        cgw_sb = wpool.tile([P, HT, 8], DT)
        v_sb = wpool.tile([P, TT, NH_LOC, HD + 1], DT)   # V + ones column
        mod_sb = wpool.tile([8, S], DT)       # sigmoid gate rows (heads)
        cwq_sb = wpool.tile([P, HT, 512], DT, tag="wA", name="cwq")
        cwk_sb = wpool.tile([P, HT, 512], DT, tag="wB", name="cwk")
        cwv_sb = wpool.tile([P, HT, 512], DT, tag="wC", name="cwv")
        cwo_sb = wpool.tile([P, 4, H], DT, tag="wD", name="cwo")

        if any(bias_on.values()):
            biasp_sb = cpool.tile([1, NBIAS], DT)
        else:
            biasp_sb = None

        # -------- prefetch DMAs, priority 0 (deps still gate reuse) --------
        # xt alone on the sync ring (earliest consumer); main weights on the
        # vector ring; causal weights on the scalar ring.
        with tc.high_priority():
            nc.gpsimd.dma_start(modb_sb[:], modb_d.ap())
            nc.gpsimd.dma_start(cgw_sb[:],
                                cgw_d.ap().rearrange("(o p) c -> p o c", p=P))
            if biasp_sb is not None:
                nc.gpsimd.dma_start(biasp_sb[:], biasp_d.ap())
            for ht in range(0, HT, 2):
                nc.sync.dma_start(xt_sb[:, ht, :], r3(xt_d)[:, ht, :])
                nc.scalar.dma_start(xt_sb[:, ht + 1, :], r3(xt_d)[:, ht + 1, :])
            nc.sync.dma_start(wv_sb[:], r3(wv_d))
            # causal weights + main q/k on the gpsimd ring: its sequencer is
            # otherwise idle, so the descriptor-generation burst is free
            # (on the scalar ring it stalls the V-proj evacuations)
            nc.gpsimd.dma_start(cwq_sb[:], r3(cwq_d))
            nc.gpsimd.dma_start(cwk_sb[:], r3(cwk_d))
            nc.gpsimd.dma_start(cwv_sb[:], r3(cwv_d))
            nc.gpsimd.dma_start(cwo_sb[:], r3(cwo_d))
            nc.gpsimd.dma_start(wq_sb[:], r3(wq_d))
            nc.gpsimd.dma_start(wk_sb[:], r3(wk_d))

        mpool = stack.enter_context(
            tc.tile_pool(name="psA", bufs=1, space="PSUM"))
        dpool = stack.enter_context(tc.tile_pool(name="dram", bufs=1, space="DRAM"))
        groups = [[0, 1], [2, 3], [4, 5], [6, 7]]

        rs1_in = dpool.tile([2, 2, 256, H], DT)   # [chunk, half, rows, cols]
        rs1_out = dpool.tile([2, 256, H], DT)
        ag_in = dpool.tile([4, 128, H], DT)
        ag_out = dpool.tile([4, 2, 128, H], DT)
        ag3_in = dpool.tile([2, 128, 512], DT)    # pair-3 AG, column halves
        ag3_out = dpool.tile([2, 2, 128, 512], DT)

        with tc.tile_pool(name="rsum", bufs=1) as rpool, \
             tc.tile_pool(name="qk", bufs=2) as qkpool, \
             tc.tile_pool(name="mexpS", bufs=2) as xpool, \
             tc.tile_pool(name="mha2", bufs=1) as mhapool:

            # mod = sigmoid(x @ cgW + modb), ht-major (consume xt as it lands)
            nc.vector.memset(v_sb[:, :, :, HD], 1.0)
            g_ps = mpool.tile([8, 2, QW], F32, tag="s2", bufs=3)
            for ht in range(HT):
                for qb in range(QB):
                    nc.tensor.matmul(g_ps[:, qb, :], cgw_sb[:, ht, :],
                                     xt_sb[:, ht, qb * QW:(qb + 1) * QW],
                                     start=(ht == 0), stop=(ht == HT - 1))
            nc.scalar.activation(
                mod_sb[:, :].rearrange("p (a b) -> p a b", a=2),
                g_ps[:], SIG, bias=modb_sb[:, 0:1], scale=1.0)

            # V projection (all 8 heads at once); evac on Scalar (idle here)
            for tt in range(TT):
                v_ps = mpool.tile([P, QW], F32, tag="pv", bufs=2)
                for ht in range(HT):
                    nc.tensor.matmul(v_ps[:],
                                     xt_sb[:, ht, tt * P:(tt + 1) * P],
                                     wv_sb[:, ht, :],
                                     start=(ht == 0),
                                     stop=(ht == HT - 1 and not bias_on["bv"]))
                if bias_on["bv"]:
                    nc.tensor.matmul(v_ps[:], ones_row[0:1, 0:P],
                                     biasp_sb[0:1, BV_OFF:BV_OFF + 512],
                                     start=False, stop=True)
                nc.vector.tensor_copy(
                    v_sb[:, tt, :, 0:HD],
                    v_ps[:, :].rearrange("p (h d) -> p h d", h=NH_LOC))

            # mod-gate broadcast rows for ALL pairs, built up front (keeps
            # the steady-state pair pipeline free of PSUM tag coupling)
            mb_all = wpool.tile([P, 4, S], DT, name="mb_all")
            for _p in range(4):
                _j0 = _p * 2
                modrow_t = qkpool.tile([2, S], DT, tag="modrow", bufs=2,
                                       name=f"mr{_p}")
                nc.sync.dma_start(modrow_t[:], mod_sb[_j0:_j0 + 2, :])
                for _qb in range(QB):
                    mb_ps = mpool.tile([P, QW], F32, tag="pv", bufs=2,
                                       name=f"mbp{_p}{_qb}")
                    nc.tensor.matmul(mb_ps[:], indic[:],
                                     modrow_t[:, _qb * QW:(_qb + 1) * QW],
                                     start=True, stop=True)
                    nc.vector.tensor_copy(
                        mb_all[:, _p, _qb * QW:(_qb + 1) * QW], mb_ps[:])

            # ---- main-attention emitters (software-pipelined) --------------
            # Per pair: proj matmuls + evacs emit together; the deferred
            # rb+mult flush of the previous block always has PE work (next
            # block's scores or next pair's projections) in front of it.
            pst = {}   # per-pair live tiles

            def emit_proj_mm(pair):
                j0 = pair * 2
                st = pst.setdefault(pair, {})
                st["qm"] = qkpool.tile([P, S], DT, tag="qm", bufs=1,
                                       name=f"qm{pair}")
                st["km"] = qkpool.tile([P, S], DT, tag="km", bufs=1,
                                       name=f"km{pair}")
                # q^T / k^T projections, both sub-heads in one 128-wide mm
                pp = []
                for dst_kind in range(2):
                    p_ps = mpool.tile([P, 2, QW], F32, tag="s2", bufs=3,
                                      name=f"pp{pair}{dst_kind}")
                    w_sb = wq_sb if dst_kind == 0 else wk_sb
                    b_on = bias_on["bq"] if dst_kind == 0 else bias_on["bk"]
                    boff = BQ_OFF if dst_kind == 0 else BK_OFF
                    for ht in range(HT):
                        for qb in range(QB):
                            nc.tensor.matmul(
                                p_ps[:, qb, :],
                                w_sb[:, ht, j0 * HD:(j0 + 2) * HD],
                                xt_sb[:, ht, qb * QW:(qb + 1) * QW],
                                start=(ht == 0),
                                stop=(ht == HT - 1 and not b_on))
                    if b_on:
                        for qb in range(QB):
                            nc.tensor.matmul(
                                p_ps[:, qb, :],
                                biasp_sb[0:1,
                                         boff + j0 * HD:boff + (j0 + 2) * HD],
                                ones_row[0:1, :], start=False, stop=True)
                    pp.append(p_ps)
                # evacuations on DVE: q gets the sigmoid gate, k is a copy
                for qb in range(QB):
                    qsl = slice(qb * QW, (qb + 1) * QW)
                    nc.vector.tensor_tensor(
                        st["qm"][:, qsl], pp[0][:, qb, :],
                        mb_all[:, pair, qsl], mult)
                    nc.vector.tensor_copy(st["km"][:, qsl],
                                          pp[1][:, qb, :])

            def emit_scores(pair, qb, ktp_range):
                """Scores + EXP for kt-pairs in ktp_range; returns expS."""
                st = pst[pair]
                qs = slice(qb * QW, (qb + 1) * QW)
                if "expS" not in st or st.get("expS_qb") != qb:
                    st["expS"] = [
                        xpool.tile([P, KT, QW], DT, tag="expS", bufs=3,
                                   name=f"ex{pair}{qb}{sub}")
                        for sub in range(2)]
                    st["expS_qb"] = qb
                expS = st["expS"]
                for kt in ktp_range:
                    s_ps = [mpool.tile([P, 2, QW], F32, tag="s2", bufs=3,
                                       name=f"sp{pair}{qb}{kt}{sub}")
                            for sub in range(2)]
                    for half in range(2):
                        for sub in range(2):
                            po = sub * 64
                            nc.tensor.matmul(
                                s_ps[sub][:, half, :],
                                st["km"][po:po + 64,
                                         (kt + half) * P:(kt + half + 1) * P],
                                st["qm"][po:po + 64, qs],
                                start=True, stop=True)
                    for sub in range(2):
                        nc.scalar.activation(expS[sub][:, kt:kt + 2, :],
                                             s_ps[sub][:], EXP, scale=0.125)

            def emit_pv_recip(pair, qb):
                """PV chains (kt-interleaved over subs) + recips on DVE."""
                st = pst[pair]
                j0 = pair * 2
                expS = st["expS"]
                c_ps = [mpool.tile([HD + 1, QW], F32, tag="pv", bufs=2,
                                   name=f"cx{pair}{qb}{sub}")
                        for sub in range(2)]
                for kt in range(KT):
                    for sub in range(2):
                        nc.tensor.matmul(c_ps[sub][:], v_sb[:, kt, j0 + sub, :],
                                         expS[sub][:, kt, :],
                                         start=(kt == 0),
                                         stop=(kt == KT - 1))
                recip16 = []
                for sub in range(2):
                    den_sb = qkpool.tile([1, QW], F32, tag="densb", bufs=2,
                                         name=f"dn{pair}{qb}{sub}")
                    nc.vector.tensor_copy(den_sb[:], c_ps[sub][64:65, :])
                    rtmp = qkpool.tile([1, QW], F32, tag="rtmp", bufs=2,
                                       name=f"rt{pair}{qb}{sub}")
                    nc.vector.reciprocal_approx_fast(rtmp[:], den_sb[:])
                    r16 = qkpool.tile([1, QW], DT, tag="recip16", bufs=2,
                                      name=f"r6{pair}{qb}{sub}")
                    nc.vector.tensor_copy(r16[:], rtmp[:])
                    recip16.append(r16)
                return {"pair": pair, "qb": qb, "c_ps": c_ps,
                        "recip16": recip16}

            def flush_rb(pend):
                """Deferred: broadcast 1/den and scale ctx into ch_sb."""
                if pend is None:
                    return
                pair, qb = pend["pair"], pend["qb"]
                qs = slice(qb * QW, (qb + 1) * QW)
                rb_ps = mpool.tile([P, QW], F32, tag="s2", bufs=3,
                                   name=f"rp{pair}{qb}")
                for sub in range(2):
                    po = sub * 64
                    nc.tensor.matmul(
                        rb_ps[po:po + 64, :], ones_row[0:1, 0:64],
                        pend["recip16"][sub][0:1, :],
                        start=True, stop=True)
                rb_sb = qkpool.tile([P, QW], DT, tag="rb", bufs=2,
                                    name=f"rs{pair}{qb}")
                nc.scalar.copy(rb_sb[:], rb_ps[:])
                for sub in range(2):
                    po = sub * 64
                    nc.vector.tensor_tensor(
                        ch_sb[po:po + 64, pair, qs],
                        pend["c_ps"][sub][0:64, :], rb_sb[po:po + 64, :], mult)

            ex_state = {}

            def emit_exchange_pre(pair):
                c = pair
                if c % 2 == 0:
                    rsum_sb = rpool.tile([P, 2, S], DT, tag="rsum", bufs=1,
                                         name=f"rsum{c}")
                    nc.sync.dma_start(
                        rsum_sb[:],
                        rs1_out[c // 2].rearrange("(o p) c -> p o c", p=P))
                    ex_state["rsum"] = rsum_sb
                nc.vector.tensor_tensor(
                    ch_sb[:, c, :], ex_state["rsum"][:, c % 2, :],
                    ch_sb[:, c, :], add)
                nc.sync.dma_start(
                    ag_in[c].rearrange("(o p) c -> p o c", p=P)[:, 0, :],
                    ch_sb[:, c, :])

            def emit_exchange_ag(pair):
                c = pair
                nc.gpsimd.collective_compute(
                    "AllGather", mybir.AluOpType.bypass,
                    replica_groups=groups,
                    ins=[ag_in[c].opt()], outs=[ag_out[c].opt()])
                # readbacks on the scalar ring so the next pair's ag_in
                # write on the sync ring is not serialized behind them
                for half in range(2):
                    nc.scalar.dma_start(
                        ctxT_sb[:, half * 4 + c, :],
                        ag_out[c, half].rearrange("(o p) c -> p o c", p=P)[:, 0, :])

            def emit_exchange_half3(h):
                # pair 3: exchange each query-half as soon as its mults land,
                # so the last AllGather is mostly off the critical path
                hs = slice(h * 512, (h + 1) * 512)
                nc.vector.tensor_tensor(
                    ch_sb[:, 3, hs], ex_state["rsum"][:, 1, hs],
                    ch_sb[:, 3, hs], add)
                nc.sync.dma_start(
                    ag3_in[h].rearrange("(o p) c -> p o c", p=P)[:, 0, :],
                    ch_sb[:, 3, hs])
                nc.gpsimd.collective_compute(
                    "AllGather", mybir.AluOpType.bypass,
                    replica_groups=groups,
                    ins=[ag3_in[h].opt()], outs=[ag3_out[h].opt()])
                for half in range(2):
                    nc.sync.dma_start(
                        ctxT_sb[:, half * 4 + 3, hs],
                        ag3_out[h, half].rearrange("(o p) c -> p o c", p=P)[:, 0, :])

            # -------- causal branch; emitted first (its EXPs overlap the ----
            # -------- main projections that follow in the PE queue) ---------
            def rs1_cb(c):
                nc.gpsimd.collective_compute(
                    "ReduceScatter", add, replica_groups=groups,
                    ins=[rs1_in[c].opt()], outs=[rs1_out[c].opt()])

            _mha256(nc, tc, mpool, mhapool, xt_sb,
                    (cwq_sb, cwk_sb, cwv_sb, cwo_sb),
                    rs1_in, ones_row, ones_col, indic, biasp_sb,
                    (CQ_OFF, CK_OFF, CV_OFF, CBO_OFF),
                    (bias_on["cq"], bias_on["ck"], bias_on["cv"],
                     bias_on["cbo"]),
                    "c", chunk_cb=rs1_cb, shared_xpool=xpool)

            # meta weights: enqueue now (slot-free sems gate the transfers,
            # but the enqueue must not sit behind the main-attention EXPs)
            mwq_sb = wpool.tile([P, HT, 512], DT, tag="wA", name="mwq")
            mwk_sb = wpool.tile([P, HT, 512], DT, tag="wB", name="mwk")
            mwv_sb = wpool.tile([P, HT, 512], DT, tag="wC", name="mwv")
            mwo_sb = wpool.tile([P, 4, H], DT, tag="wD", name="mwo")
            with tc.high_priority():
                nc.scalar.dma_start(mwq_sb[:], r3(mwq_d))
                nc.scalar.dma_start(mwk_sb[:], r3(mwk_d))
                nc.scalar.dma_start(mwv_sb[:], r3(mwv_d))
                nc.scalar.dma_start(mwo_sb[:], r3(mwo_d))

            # main attention, software-pipelined; AG per pair
            pending = None
            emit_proj_mm(0)
            for _pair in range(4):
                for _qb in range(QB):
                    emit_scores(_pair, _qb, [0, 2, 4])
                    was_qb0 = pending is not None and pending["qb"] == 0
                    flush_rb(pending)
                    pending = None
                    if _pair == 3 and was_qb0:
                        emit_exchange_half3(0)
                    emit_scores(_pair, _qb, [6])
                    pending = emit_pv_recip(_pair, _qb)
                if _pair < 3:
                    emit_proj_mm(_pair + 1)
                    flush_rb(pending)
                    pending = None
                    emit_exchange_pre(_pair)
                    emit_exchange_ag(_pair)
                else:
                    flush_rb(pending)
                    pending = None
                    emit_exchange_half3(1)

            # ===== meta branch: full in-projections over AllGathered ctx ====
            # Chains are emitted as a wavefront (3 at a time, early ctxT
            # chunks first) so the PE fills the last AllGather's latency
            # instead of head-of-line blocking on chunks 3/7.
            # meta-branch tiles reuse the causal branch's slots (now dead)
            m_qcT = mhapool.tile([P, 4, S], DT, tag="qcT", name="m_qcT")
            m_kcT = mhapool.tile([P, 4, S], DT, tag="kcT", name="m_kcT")
            m_vc = mhapool.tile([P, TT, 512], DT, tag="vc", name="m_vc")
            m_attnT = mhapool.tile([P, 4, S], DT, tag="attnT", name="m_attnT")

            # 8 q/k chains: (dst, dc) pairs, grouped 3-wide over PSUM slots
            qk_chains = [(di, dc) for dc in range(4) for di in range(2)]
            for g0 in range(0, len(qk_chains), 3):
                grp = qk_chains[g0:g0 + 3]
                tiles = {}
                for di, dc in grp:
                    tiles[(di, dc)] = mpool.tile(
                        [P, 2, QW], F32, tag="s2", bufs=3,
                        name=f"mpp{di}{dc}")
                steps = ([(hi, ht, qb) for hi, ht in enumerate(HT_ORDER[:6])
                          for qb in range(QB)]
                         + [(6 + i, HT_ORDER[6 + i], qb)
                            for qb in range(QB) for i in range(2)])
                for hi, ht, qb in steps:
                    for di, dc in grp:
                        w_sb = mwq_sb if di == 0 else mwk_sb
                        b_on = bias_on["mq"] if di == 0 else bias_on["mk"]
                        nc.tensor.matmul(
                            tiles[(di, dc)][:, qb, :],
                            w_sb[:, ht, dc * P:(dc + 1) * P],
                            ctxT_sb[:, ht, qb * QW:(qb + 1) * QW],
                            start=(hi == 0),
                            stop=(hi == HT - 1 and not b_on))
                for di, dc in grp:
                    b_on = bias_on["mq"] if di == 0 else bias_on["mk"]
                    boff = MQ_OFF if di == 0 else MK_OFF
                    if b_on:
                        for qb in range(QB):
                            nc.tensor.matmul(
                                tiles[(di, dc)][:, qb, :],
                                biasp_sb[0:1, boff + dc * P:boff + (dc + 1) * P],
                                ones_row[0:1, :], start=False, stop=True)
                    dst = m_qcT if di == 0 else m_kcT
                    nc.scalar.copy(
                        dst[:, dc, :].rearrange("p (a b) -> p a b", a=2),
                        tiles[(di, dc)])
            # v chains, 2-wide
            for t0 in range(0, TT, 2):
                vt = {}
                for tt in range(t0, t0 + 2):
                    vt[tt] = mpool.tile([P, QW], F32, tag="pv", bufs=2,
                                        name=f"mv{tt}")
                for hi, ht in enumerate(HT_ORDER):
                    for tt in range(t0, t0 + 2):
                        nc.tensor.matmul(
                            vt[tt][:],
                            ctxT_sb[:, ht, tt * P:(tt + 1) * P],
                            mwv_sb[:, ht, :],
                            start=(hi == 0),
                            stop=(hi == HT - 1 and not bias_on["mv"]))
                for tt in range(t0, t0 + 2):
                    if bias_on["mv"]:
                        nc.tensor.matmul(vt[tt][:], ones_row[0:1, 0:P],
                                         biasp_sb[0:1, MV_OFF:MV_OFF + 512],
                                         start=False, stop=True)
                    nc.scalar.copy(m_vc[:, tt, :], vt[tt][:])

            # final-Wo weights: enqueue ahead of the meta-attention EXPs
            wo_a = wpool.tile([P, 4, H], DT, tag="wA", name="wo_a")
            wo_b = wpool.tile([P, 4, H], DT, tag="wB", name="wo_b")
            with tc.high_priority():
                nc.scalar.dma_start(wo_a[:], r3(wo_d)[:, 0:4, :])
                nc.scalar.dma_start(wo_b[:], r3(wo_d)[:, 4:8, :])

            # meta attention + out-projection (kept local; summed on host)
            mp_sb = apool.tile([P, HT, S], DT, tag="xt_mp")
            _attn256(nc, tc, mpool, (m_qcT, m_kcT, m_vc, m_attnT), mwo_sb,
                     mp_sb, ones_row, ones_col, biasp_sb,
                     MBO_OFF, bias_on["mbo"], "c", out_sb=True,
                     xpool=xpool)

        # ===== Z = 0.425*ctx + mp_local ; out_partial = Z^T.T @ Wo (full) ====
        # wo reuses the (dead) meta in-proj slots; final matmuls consume the
        # blended chunks in meta out-proj emission order (PT_ORDER).
        with tc.tile_pool(name="fin", bufs=1) as fpool, \
             tc.tile_pool(name="fstage", bufs=3) as spool:
            zs_sb = fpool.tile([P, HT, S], DT)    # final blended Z^T
            for pt in PT_ORDER:
                nc.vector.scalar_tensor_tensor(zs_sb[:, pt, :],
                                               ctxT_sb[:, pt, :], 0.425,
                                               mp_sb[:, pt, :], mult, add)
            for tt in range(TT):
                o_ps = mpool.tile([P, 2, QW], F32, tag="s2", bufs=3)
                for cb in range(2):
                    for ci, ct in enumerate(PT_ORDER):
                        wo_sb = wo_a if ct < 4 else wo_b
                        nc.tensor.matmul(o_ps[:, cb, :],
                                         zs_sb[:, ct, tt * P:(tt + 1) * P],
                                         wo_sb[:, ct % 4, cb * 512:(cb + 1) * 512],
                                         start=(ci == 0), stop=(ci == HT - 1))
                o_sb = spool.tile([P, 2, QW], DT, tag="outst")
                if tt % 2 == 0:
                    nc.vector.tensor_copy(o_sb[:], o_ps[:])
                else:
                    nc.scalar.copy(o_sb[:], o_ps[:])
                nc.sync.dma_start(out_v[:, tt, :],
                                  o_sb[:, :, :].rearrange("p a b -> p (a b)"))


def _mha256(nc, tc, mpool, mhapool, x_sb, w_tiles, out_dram,
            ones_row, ones_col, indic, biasp_sb, boffs, bflags, prefix,
            chunk_cb=None, shared_xpool=None):
    """Causal 256-dim-head MHA branch: full in-proj + attention + out-proj.

    x_sb     [P, HT, S]  input ^T
    w_tiles  (wq, wk, wv, wo) SBUF tiles, pre-DMA'd by the caller:
             wq/k/v [P, HT, 512] in-proj slices (my 2 heads),
             wo [P, 4, H] out-proj rows slice (pre-scaled by blend weight)
    out_dram [2, 2, 256, H] dram bounce for the chunked ReduceScatter
    chunk_cb(c) is invoked right after chunk c's out-proj tiles are emitted
    """
    qoff, koff, voff, booff = boffs
    bq_on, bk_on, bv_on, bo_on = bflags

    wq_sb, wk_sb, wv_sb, wo_sb = w_tiles

    import contextlib
    _st = contextlib.ExitStack()
    qcT = mhapool.tile([P, 4, S], DT, tag="qcT", name=f"{prefix}_qcT")
    kcT = mhapool.tile([P, 4, S], DT, tag="kcT", name=f"{prefix}_kcT")
    vc = mhapool.tile([P, TT, 512], DT, tag="vc", name=f"{prefix}_vc")
    attnT = mhapool.tile([P, 4, S], DT, tag="attnT", name=f"{prefix}_attnT")
    if shared_xpool is None:
        xpool = _st.enter_context(tc.tile_pool(name=f"{prefix}exp", bufs=2))
    else:
        xpool = shared_xpool

    # in-projections q^T, k^T  (4 chunks of 128 rows = 2 heads x 2)
    for dc in range(4):
        for dst, w_sb, boff, b_on in ((qcT, wq_sb, qoff, bq_on),
                                      (kcT, wk_sb, koff, bk_on)):
            p_ps = mpool.tile([P, 2, QW], F32, tag="s2", bufs=3)
            for ht in range(HT):
                for qb in range(QB):
                    nc.tensor.matmul(p_ps[:, qb, :],
                                     w_sb[:, ht, dc * P:(dc + 1) * P],
                                     x_sb[:, ht, qb * QW:(qb + 1) * QW],
                                     start=(ht == 0),
                                     stop=(ht == HT - 1 and not b_on))
            if b_on:
                for qb in range(QB):
                    nc.tensor.matmul(
                        p_ps[:, qb, :],
                        biasp_sb[0:1, boff + dc * P:boff + (dc + 1) * P],
                        ones_row[0:1, :], start=False, stop=True)
            nc.vector.tensor_copy(
                dst[:, dc, :].rearrange("p (a b) -> p a b", a=2),
                p_ps[:])
    # v (normal layout)
    for tt in range(TT):
        v_ps = mpool.tile([P, QW], F32, tag="pv", bufs=2)
        for ht in range(HT):
            nc.tensor.matmul(v_ps[:], x_sb[:, ht, tt * P:(tt + 1) * P],
                             wv_sb[:, ht, :],
                             start=(ht == 0),
                             stop=(ht == HT - 1 and not bv_on))
        if bv_on:
            nc.tensor.matmul(v_ps[:], ones_row[0:1, 0:P],
                             biasp_sb[0:1, voff:voff + 512],
                             start=False, stop=True)
        nc.vector.tensor_copy(vc[:, tt, :], v_ps[:])

    _attn256(nc, tc, mpool, (qcT, kcT, vc, attnT), wo_sb, out_dram,
             ones_row, ones_col, biasp_sb, booff, bo_on, prefix,
             chunk_cb=chunk_cb, out_sb=False, xpool=xpool)
    _st.close()


def _attn256(nc, tc, mpool, t_tiles, wo_sb, out_dram,
             ones_row, ones_col, biasp_sb, booff, bo_on, prefix,
             chunk_cb=None, out_sb=False, xpool=None):
    """Attention + out-projection for the 256-dim-head branches.

    Per block: scores -> EXP -> den matmuls -> PV chains (the den->recip
    roundtrip runs on DVE underneath the PV chains) -> rb broadcast ->
    normalizing mults (DVE)."""
    mult = mybir.AluOpType.mult
    EXP = mybir.ActivationFunctionType.Exp
    qcT, kcT, vc, attnT = t_tiles

    # attention per head
    for jc in range(CH_LOC):
        for qb in range(QB):
            qs = slice(qb * QW, (qb + 1) * QW)
            expS = xpool.tile([P, KT, QW], DT, tag="expS", bufs=3)
            for kt in range(0, KT, 2):
                s_ps = mpool.tile([P, 2, QW], F32, tag="s2", bufs=3)
                for half in range(2):
                    for dc in range(2):
                        nc.tensor.matmul(
                            s_ps[:, half, :],
                            kcT[:, jc * 2 + dc, (kt + half) * P:(kt + half + 1) * P],
                            qcT[:, jc * 2 + dc, qs],
                            start=(dc == 0), stop=(dc == 1))
                nc.scalar.activation(expS[:, kt:kt + 2, :], s_ps[:], EXP,
                                     scale=0.0625)
            den_ps = mpool.tile([1, QW], F32, tag="pv", bufs=2)
            for kt in range(KT):
                nc.tensor.matmul(den_ps[:], ones_col[:, 0:1], expS[:, kt, :],
                                 start=(kt == 0), stop=(kt == KT - 1))
            # recip path on DVE, overlapped with the PV chains below
            den_row = xpool.tile([1, QW], F32, tag=f"{prefix}denrow", bufs=1)
            nc.vector.tensor_copy(den_row[:], den_ps[:])
            recip = xpool.tile([1, QW], F32, tag=f"{prefix}recip", bufs=1)
            nc.vector.reciprocal_approx_fast(recip[:], den_row[:])
            recip16 = xpool.tile([1, QW], DT, tag=f"{prefix}recip16", bufs=1)
            nc.vector.tensor_copy(recip16[:], recip[:])
            pv_ps = []
            for dc in range(2):
                c_ps = mpool.tile([P, QW], F32, tag="pv", bufs=2)
                for kt in range(KT):
                    nc.tensor.matmul(c_ps[:],
                                     vc[:, kt, (jc * 2 + dc) * P:(jc * 2 + dc + 1) * P],
                                     expS[:, kt, :],
                                     start=(kt == 0), stop=(kt == KT - 1))
                pv_ps.append(c_ps)
            rb_ps = mpool.tile([P, QW], F32, tag="s2", bufs=3)
            nc.tensor.matmul(rb_ps[:], ones_row[0:1, 0:P], recip16[:],
                             start=True, stop=True)
            rb_sb = xpool.tile([P, QW], DT, tag=f"{prefix}rb", bufs=1)
            nc.vector.tensor_copy(rb_sb[:], rb_ps[:])
            for dc in range(2):
                nc.vector.tensor_tensor(attnT[:, jc * 2 + dc, qs],
                                        pv_ps[dc][:], rb_sb[:], mult)

    # out-projection: [512 local dims] x [H out rows], chunk-pipelined order,
    # staged straight to the collective's dram bounce buffer
    for pi, pt in enumerate(PT_ORDER):
        c, half, r = (pt % 4) // 2, pt // 4, pt % 2
        op_ps = mpool.tile([P, 2, QW], F32, tag="s2", bufs=3)
        for qb in range(QB):
            for ct in range(4):
                nc.tensor.matmul(op_ps[:, qb, :],
                                 wo_sb[:, ct, pt * P:(pt + 1) * P],
                                 attnT[:, ct, qb * QW:(qb + 1) * QW],
                                 start=(ct == 0),
                                 stop=(ct == 3 and not bo_on))
            if bo_on:
                nc.tensor.matmul(
                    op_ps[:, qb, :],
                    biasp_sb[0:1, booff + pt * P:booff + (pt + 1) * P],
                    ones_row[0:1, :], start=False, stop=True)
        if out_sb:
            nc.scalar.copy(
                out_dram[:, pt, :].rearrange("p (a b) -> p a b", a=2), op_ps[:])
        else:
            ost = xpool.tile([P, 2, QW], DT, tag=f"{prefix}ost", bufs=2)
            nc.vector.tensor_copy(ost[:], op_ps[:])
            nc.sync.dma_start(
                out_dram[c, half].rearrange("(o p) c -> p o c", p=P)[:, r, :],
                ost[:, :, :].rearrange("p a b -> p (a b)"))
        if chunk_cb is not None and pi == 3:
            chunk_cb(0)
    if chunk_cb is not None:
        chunk_cb(1)


_CACHE = {}


def _get_compiled(bias_key):
    if bias_key in _CACHE:
        return _CACHE[bias_key]
    bias_on = dict(bias_key)
    nc = bacc.Bacc("TRN2", target_bir_lowering=False, debug=False,
                   num_devices=N_CORES)
    with tile.TileContext(nc) as tc:
        _emit(nc, tc, bias_on)
    nc.compile()
    _CACHE[bias_key] = nc
    return nc


def _bias_key(inp):
    bq, bk, bv = inp["bq"], inp["bk"], inp["bv"]
    ca_bin, ca_bout = inp["ca_bin"], inp["ca_bout"]
    mc_bin, mc_bout = inp["mc_bin"], inp["mc_bout"]
    bias_on = {
        "bq": bool(np.any(bq)), "bk": bool(np.any(bk)), "bv": bool(np.any(bv)),
        "cq": bool(np.any(ca_bin[:H])), "ck": bool(np.any(ca_bin[H:2 * H])),
        "cv": bool(np.any(ca_bin[2 * H:])), "cbo": bool(np.any(ca_bout)),
        "mq": bool(np.any(mc_bin[:H])), "mk": bool(np.any(mc_bin[H:2 * H])),
        "mv": bool(np.any(mc_bin[2 * H:])), "mbo": bool(np.any(mc_bout)),
    }
    return tuple(sorted(bias_on.items()))


def _shard_in_maps(inp):
    CAUSAL_W = 0.7
    META_W = ((0.9 - 0.8) / 0.2) * 0.3
    hidden_states = inp["hidden_states"]
    cons_vec, am_W, am_b = inp["cons_vec"], inp["am_W"], inp["am_b"]
    cg_W, cg_b = inp["cg_W"], inp["cg_b"]
    Wq, bq, Wk, bk, Wv, bv = (inp["Wq"], inp["bq"], inp["Wk"], inp["bk"],
                              inp["Wv"], inp["bv"])
    ca_Win, ca_bin, ca_Wout, ca_bout = (inp["ca_Win"], inp["ca_bin"],
                                        inp["ca_Wout"], inp["ca_bout"])
    mc_Win, mc_bin, mc_Wout, mc_bout = (inp["mc_Win"], inp["mc_bin"],
                                        inp["mc_Wout"], inp["mc_bout"])
    Wo = inp["Wo"]

    hs = np.asarray(hidden_states, np.float32)
    am_vec = np.asarray(cons_vec, np.float32) @ np.asarray(am_W, np.float32) \
        + np.asarray(am_b, np.float32)
    modb_full = np.asarray(cg_b, np.float32) + am_vec          # [16]

    def b16(a):
        return np.ascontiguousarray(np.asarray(a, np.float32)).astype(BF16)

    in_maps = []
    for c in range(N_CORES):
        b, h = c // 2, c % 2
        cols = slice(h * 512, (h + 1) * 512)
        rows_own = slice(h * 512, (h + 1) * 512)
        biasp = np.zeros(NBIAS, np.float32)
        biasp[BQ_OFF:BQ_OFF + 512] = np.asarray(bq, np.float32)[cols]
        biasp[BK_OFF:BK_OFF + 512] = np.asarray(bk, np.float32)[cols]
        biasp[BV_OFF:BV_OFF + 512] = 0.3 * np.asarray(bv, np.float32)[cols]
        biasp[CQ_OFF:CQ_OFF + 512] = np.asarray(ca_bin, np.float32)[0:H][cols]
        biasp[CK_OFF:CK_OFF + 512] = np.asarray(ca_bin, np.float32)[H:2 * H][cols]
        biasp[CV_OFF:CV_OFF + 512] = np.asarray(ca_bin, np.float32)[2 * H:][cols]
        biasp[MQ_OFF:MQ_OFF + 512] = np.asarray(mc_bin, np.float32)[0:H][cols]
        biasp[MK_OFF:MK_OFF + 512] = np.asarray(mc_bin, np.float32)[H:2 * H][cols]
        biasp[MV_OFF:MV_OFF + 512] = np.asarray(mc_bin, np.float32)[2 * H:][cols]
        if h == 0:
            biasp[CBO_OFF:CBO_OFF + H] = CAUSAL_W * np.asarray(ca_bout, np.float32)
            biasp[MBO_OFF:MBO_OFF + H] = META_W * np.asarray(mc_bout, np.float32)
        m = {
            "xt": b16(hs[b].T),
            "wq": b16(Wq[:, cols]),
            "wk": b16(Wk[:, cols]),
            "wv": b16(0.3 * np.asarray(Wv, np.float32)[:, cols]),
            "cgw": b16(np.asarray(cg_W, np.float32)[:, h * 8:(h + 1) * 8]),
            "modb": np.ascontiguousarray(
                modb_full[h * 8:(h + 1) * 8].reshape(8, 1)),
            "cwq": b16(np.asarray(ca_Win, np.float32)[:, 0:H][:, cols]),
            "cwk": b16(np.asarray(ca_Win, np.float32)[:, H:2 * H][:, cols]),
            "cwv": b16(np.asarray(ca_Win, np.float32)[:, 2 * H:][:, cols]),
            "cwo": b16(CAUSAL_W * np.asarray(ca_Wout, np.float32)[cols, :]),
            "mwq": b16(np.asarray(mc_Win, np.float32)[:, 0:H][:, cols]),
            "mwk": b16(np.asarray(mc_Win, np.float32)[:, H:2 * H][:, cols]),
            "mwv": b16(np.asarray(mc_Win, np.float32)[:, 2 * H:][:, cols]),
            "mwo": b16(META_W * np.asarray(mc_Wout, np.float32)[cols, :]),
            "wo": b16(np.asarray(Wo, np.float32)),
            "biasp": biasp.reshape(1, NBIAS).astype(BF16),
        }
        in_maps.append(m)
    return in_maps


def kernel(**inputs):
    _install_ntff_hook()
    nc = _get_compiled(_bias_key(inputs))
    in_maps = _shard_in_maps(inputs)
    res = None
    for attempt in range(3):
        try:
            res = bass_utils.run_bass_kernel_spmd(nc, in_maps,
                                                  core_ids=list(range(N_CORES)))
            break
        except Exception:
            if attempt == 2:
                raise
            import time as _time
            _time.sleep(2.0)
    out = np.zeros((4, S, H), np.float32)
    bo_f = np.asarray(inputs["bo"], np.float32)
    for b in range(4):
        out[b] = (np.asarray(res.results[2 * b]["out"], np.float32)
                  + np.asarray(res.results[2 * b + 1]["out"], np.float32)
                  + bo_f)
    return out


# revision 10
# speedup vs baseline: 1.0360x; 1.0360x over previous
"""Trainium2 Bass kernel for nn_AGITransformerLayer (B=4, S=1024, H=1024, NH=16).

Distribution over 8 NeuronCores: data-parallel over the 4 batches x 2-way
tensor-parallel within each adjacent core pair (cores 2b, 2b+1 handle batch b).
Within a pair, core h (=0,1) owns:
  - main attention heads h*8..h*8+8  (16 heads of dim 64, split 8/8)
  - causal-MHA heads h*2..h*2+2      (4 heads of dim 256, split 2/2)
  - meta-MHA heads h*2..h*2+2        (4 heads of dim 256, split 2/2)
  - contraction rows h*512..h*512+512 of the causal/meta out-projections and
    of the final Wo projection.

Everything on chip lives in "transposed" layout [feature, token] so every
matmul uses natural (un-transposed) operands.  Cross-core exchange per pair
(all chunked 2-way and pipelined against compute):
  ReduceScatter(causal out-proj partials) -> + local main-attn ctx half
  meta in-projections run on the LOCAL blended half only (the host hands each
  core the matching 512 rows of mc_Win) and the q/k/v partials are summed
  with an AllReduce -- so the last AllGather is off the critical path and is
  consumed only by the final residual blend.
  AllGather(ctx half) -> full blended ctx for the final blend.
The pair's two final-output partials are summed on the host.

Engine balance: Scalar(ACT) does softmax EXPs only (it is the 2nd roofline at
~100us); all PSUM evacuations run on DVE (or Scalar when it is idle: V-proj,
meta in-proj, tail).  The softmax den->recip->broadcast roundtrip is emitted
deferred (main) / under the PV chain (causal+meta) so the PE never waits on
it.  Q/K projections use full 128-wide stationaries.

Compute dtype bf16 (fp32 PSUM accumulation); softmax denominators via an
appended ones-column on V (main heads) / explicit ones-matmuls (256-dim
heads); normalizers broadcast via indicator-matrix matmuls.
"""

import sys
import types

if "/opt/trn_rl_repo" not in sys.path:
    sys.path.insert(0, "/opt/trn_rl_repo")

import numpy as np
import ml_dtypes

import concourse.bass as bass
import concourse.tile as tile
from concourse import bacc, mybir
from concourse import bass_utils

BF16 = ml_dtypes.bfloat16
P = 128
S = 1024          # sequence length
H = 1024          # hidden dim
NH_LOC = 8        # main heads per core
HD = 64           # main head dim
CH_LOC = 2        # causal/meta heads per core
CHD = 256         # causal/meta head dim
HT = H // P       # hidden tiles (8)
TT = S // P       # token tiles (8)
QB = 2            # query blocks of 512
QW = 512          # query block width
KT = S // P       # key tiles (8)
N_CORES = 8

DT = mybir.dt.bfloat16
F32 = mybir.dt.float32

# bias-pack offsets (bf16 row [1, NBIAS]); meta q/k/v slots are 1024 wide
# (biases for BOTH cores' heads -- they enter once via core 0's partials)
BQ_OFF, BK_OFF, BV_OFF = 0, 512, 1024
CQ_OFF, CK_OFF, CV_OFF = 1536, 2048, 2560
CBO_OFF = 3072
MQ_OFF, MK_OFF, MV_OFF = 4096, 5120, 6144
MBO_OFF = 7168
NBIAS = 8192

# out-proj row-tile emission order: chunk 0 = global rows 0:256 & 512:768
PT_ORDER = [0, 1, 4, 5, 2, 3, 6, 7]
# ctxT chunk consumption order = AllGather arrival order (pair 3 last)
HT_ORDER = [0, 4, 1, 5, 2, 6, 3, 7]


def _install_ntff_hook():
    """Make trace=True work under axon (inject missing antenv.axon_hooks)."""
    if "antenv.axon_hooks" in sys.modules:
        return
    try:
        mod = types.ModuleType("antenv.axon_hooks")
        mod._hook = None
        mod.set_axon_ntff_profile_hook = lambda h: setattr(mod, "_hook", h)
        mod.get_axon_ntff_profile_hook = lambda: mod._hook
        import antenv
        antenv.axon_hooks = mod
        sys.modules["antenv.axon_hooks"] = mod
        from trn_agent_boot.trn_boot import _ntff_profile_via_ctypes
        mod.set_axon_ntff_profile_hook(
            _ntff_profile_via_ctypes("/opt/axon/libaxon_pjrt.so"))
        bass_utils.upload_artifacts = lambda tmpdir: tmpdir
    except Exception:
        pass



def _emit(nc, tc, bias_on):
    """Emit the whole per-core program.  bias_on: dict of bools (graph-uniform)."""
    xt_d = nc.dram_tensor("xt", [H, S], DT, kind="ExternalInput")
    wq_d = nc.dram_tensor("wq", [H, 512], DT, kind="ExternalInput")
    wk_d = nc.dram_tensor("wk", [H, 512], DT, kind="ExternalInput")
    wv_d = nc.dram_tensor("wv", [H, 512], DT, kind="ExternalInput")
    cgw_d = nc.dram_tensor("cgw", [H, 8], DT, kind="ExternalInput")
    modb_d = nc.dram_tensor("modb", [8, 1], F32, kind="ExternalInput")
    cwq_d = nc.dram_tensor("cwq", [H, 512], DT, kind="ExternalInput")
    cwk_d = nc.dram_tensor("cwk", [H, 512], DT, kind="ExternalInput")
    cwv_d = nc.dram_tensor("cwv", [H, 512], DT, kind="ExternalInput")
    cwo_d = nc.dram_tensor("cwo", [512, H], DT, kind="ExternalInput")
    mwq_d = nc.dram_tensor("mwq", [H, 512], DT, kind="ExternalInput")
    mwk_d = nc.dram_tensor("mwk", [H, 512], DT, kind="ExternalInput")
    mwv_d = nc.dram_tensor("mwv", [H, 512], DT, kind="ExternalInput")
    mwo_d = nc.dram_tensor("mwo", [512, H], DT, kind="ExternalInput")
    wo_d = nc.dram_tensor("wo", [H, H], DT, kind="ExternalInput")
    biasp_d = nc.dram_tensor("biasp", [1, NBIAS], DT, kind="ExternalInput")
    out_d = nc.dram_tensor("out", [S, H], DT, kind="ExternalOutput")

    def r3(d):  # [R, C] dram -> [P, R//P, C] view
        return d.ap().rearrange("(o p) c -> p o c", p=P)

    out_v = out_d.ap().rearrange("(o p) c -> p o c", p=P)

    mult, add = mybir.AluOpType.mult, mybir.AluOpType.add
    EXP = mybir.ActivationFunctionType.Exp
    SIG = mybir.ActivationFunctionType.Sigmoid

    import contextlib
    stack = contextlib.ExitStack()
    with stack:
        cpool = stack.enter_context(tc.tile_pool(name="const", bufs=1))
        ones_row = cpool.tile([1, 512], DT)
        nc.vector.memset(ones_row[:], 1.0)
        ones_col = cpool.tile([P, 1], DT)
        nc.vector.memset(ones_col[:], 1.0)
        # indicator: row 0 -> out partitions 0:64, row 1 -> 64:128
        # (built on partition 0 and reshaped into 2 partitions via DMA --
        # engines may not address a single partition above 0 directly)
        indic_st = cpool.tile([1, 2 * P], DT)
        nc.vector.memset(indic_st[:], 0.0)
        nc.vector.memset(indic_st[0:1, 0:64], 1.0)
        nc.vector.memset(indic_st[0:1, P + 64:2 * P], 1.0)
        indic = cpool.tile([2, P], DT)
        nc.sync.dma_start(indic[:], indic_st[0:1, :])
        modb_sb = cpool.tile([8, 1], F32)

        # persistent across stages
        apool = stack.enter_context(tc.tile_pool(name="persist", bufs=1))
        xt_sb = apool.tile([P, HT, S], DT, tag="xt_mp")
        ch_sb = apool.tile([P, 4, S], DT)      # blended ctx^T, my half rows
        ctxT_sb = apool.tile([P, HT, S], DT)

        # all weights, prefetched; causal->meta->wo share slots via tags
        wpool = stack.enter_context(tc.tile_pool(name="wall", bufs=1))
        wq_sb = wpool.tile([P, HT, 512], DT)
        wk_sb = wpool.tile([P, HT, 512], DT)
        wv_sb = wpool.tile([P, HT, 512], DT)
        cgw_sb = wpool.tile([P, HT, 8], DT)
        v_sb = wpool.tile([P, TT, NH_LOC, HD + 1], DT)   # V + ones column
        mod_sb = wpool.tile([8, S], DT)       # sigmoid gate rows (heads)
        cwq_sb = wpool.tile([P, HT, 512], DT, tag="wA", name="cwq")
        cwk_sb = wpool.tile([P, HT, 512], DT, tag="wB", name="cwk")
        cwv_sb = wpool.tile([P, HT, 512], DT, tag="wC", name="cwv")
        cwo_sb = wpool.tile([P, 4, H], DT, tag="wD", name="cwo")

        if any(bias_on.values()):
            biasp_sb = cpool.tile([1, NBIAS], DT)
        else:
            biasp_sb = None

        # -------- prefetch DMAs, priority 0 (deps still gate reuse) --------
        # xt alone on the sync ring (earliest consumer); main weights on the
        # vector ring; causal weights on the scalar ring.
        with tc.high_priority():
            nc.gpsimd.dma_start(modb_sb[:], modb_d.ap())
            nc.gpsimd.dma_start(cgw_sb[:],
                                cgw_d.ap().rearrange("(o p) c -> p o c", p=P))
            if biasp_sb is not None:
                nc.gpsimd.dma_start(biasp_sb[:], biasp_d.ap())
            nc.sync.dma_start(xt_sb[:, 0, :], r3(xt_d)[:, 0, :])
            nc.sync.dma_start(wv_sb[:], r3(wv_d))
            for ht in range(2, HT, 2):
                nc.sync.dma_start(xt_sb[:, ht, :], r3(xt_d)[:, ht, :])
            for ht in range(1, HT, 2):
                nc.scalar.dma_start(xt_sb[:, ht, :], r3(xt_d)[:, ht, :])
            # causal weights + main q/k on the gpsimd ring: its sequencer is
            # otherwise idle, so the descriptor-generation burst is free
            # (on the scalar ring it stalls the V-proj evacuations)
            nc.gpsimd.dma_start(cwq_sb[:], r3(cwq_d))
            nc.gpsimd.dma_start(cwk_sb[:], r3(cwk_d))
            nc.gpsimd.dma_start(cwv_sb[:], r3(cwv_d))
            nc.gpsimd.dma_start(cwo_sb[:], r3(cwo_d))
            nc.gpsimd.dma_start(wq_sb[:], r3(wq_d))
            nc.gpsimd.dma_start(wk_sb[:], r3(wk_d))

        mpool = stack.enter_context(
            tc.tile_pool(name="psA", bufs=1, space="PSUM"))
        dpool = stack.enter_context(tc.tile_pool(name="dram", bufs=1, space="DRAM"))
        groups = [[0, 1], [2, 3], [4, 5], [6, 7]]

        rs1_in = dpool.tile([2, 2, 256, H], DT)   # [chunk, half, rows, cols]
        rs1_out = dpool.tile([2, 256, H], DT)
        ag_in = dpool.tile([4, 128, H], DT)
        ag_out = dpool.tile([4, 2, 128, H], DT)
        ag3_in = dpool.tile([2, 128, 512], DT)    # pair-3 AG, column halves
        ag3_out = dpool.tile([2, 2, 128, 512], DT)

        with tc.tile_pool(name="rsum", bufs=1) as rpool, \
             tc.tile_pool(name="qk", bufs=2) as qkpool, \
             tc.tile_pool(name="mexpS", bufs=2) as xpool, \
             tc.tile_pool(name="mha2", bufs=1) as mhapool:

            # mod = sigmoid(x @ cgW + modb), ht-major (consume xt as it lands)
            nc.vector.memset(v_sb[:, :, :, HD], 1.0)
            g_ps = mpool.tile([8, 2, QW], F32, tag="s2", bufs=3)
            for ht in range(HT):
                for qb in range(QB):
                    nc.tensor.matmul(g_ps[:, qb, :], cgw_sb[:, ht, :],
                                     xt_sb[:, ht, qb * QW:(qb + 1) * QW],
                                     start=(ht == 0), stop=(ht == HT - 1))
            nc.scalar.activation(
                mod_sb[:, :].rearrange("p (a b) -> p a b", a=2),
                g_ps[:], SIG, bias=modb_sb[:, 0:1], scale=1.0)

            # V projection (all 8 heads at once); evac on Scalar (idle here)
            for tt in range(TT):
                v_ps = mpool.tile([P, QW], F32, tag="pv", bufs=2)
                for ht in range(HT):
                    nc.tensor.matmul(v_ps[:],
                                     xt_sb[:, ht, tt * P:(tt + 1) * P],
                                     wv_sb[:, ht, :],
                                     start=(ht == 0),
                                     stop=(ht == HT - 1 and not bias_on["bv"]))
                if bias_on["bv"]:
                    nc.tensor.matmul(v_ps[:], ones_row[0:1, 0:P],
                                     biasp_sb[0:1, BV_OFF:BV_OFF + 512],
                                     start=False, stop=True)
                nc.vector.tensor_copy(
                    v_sb[:, tt, :, 0:HD],
                    v_ps[:, :].rearrange("p (h d) -> p h d", h=NH_LOC))

            # mod-gate broadcast rows for ALL pairs, built up front (keeps
            # the steady-state pair pipeline free of PSUM tag coupling)
            mb_all = wpool.tile([P, 4, S], DT, name="mb_all")
            for _p in range(4):
                _j0 = _p * 2
                modrow_t = qkpool.tile([2, S], DT, tag="modrow", bufs=2,
                                       name=f"mr{_p}")
                nc.sync.dma_start(modrow_t[:], mod_sb[_j0:_j0 + 2, :])
                for _qb in range(QB):
                    mb_ps = mpool.tile([P, QW], F32, tag="pv", bufs=2,
                                       name=f"mbp{_p}{_qb}")
                    nc.tensor.matmul(mb_ps[:], indic[:],
                                     modrow_t[:, _qb * QW:(_qb + 1) * QW],
                                     start=True, stop=True)
                    nc.vector.tensor_copy(
                        mb_all[:, _p, _qb * QW:(_qb + 1) * QW], mb_ps[:])

            # ---- main-attention emitters (software-pipelined) --------------
            # Per pair: proj matmuls + evacs emit together; the deferred
            # rb+mult flush of the previous block always has PE work (next
            # block's scores or next pair's projections) in front of it.
            pst = {}   # per-pair live tiles

            def emit_proj_mm(pair):
                j0 = pair * 2
                st = pst.setdefault(pair, {})
                st["qm"] = qkpool.tile([P, S], DT, tag="qm", bufs=1,
                                       name=f"qm{pair}")
                st["km"] = qkpool.tile([P, S], DT, tag="km", bufs=1,
                                       name=f"km{pair}")
                # q^T / k^T projections, both sub-heads in one 128-wide mm
                pp = []
                for dst_kind in range(2):
                    p_ps = mpool.tile([P, 2, QW], F32, tag="s2", bufs=3,
                                      name=f"pp{pair}{dst_kind}")
                    w_sb = wq_sb if dst_kind == 0 else wk_sb
                    b_on = bias_on["bq"] if dst_kind == 0 else bias_on["bk"]
                    boff = BQ_OFF if dst_kind == 0 else BK_OFF
                    for ht in range(HT):
                        for qb in range(QB):
                            nc.tensor.matmul(
                                p_ps[:, qb, :],
                                w_sb[:, ht, j0 * HD:(j0 + 2) * HD],
                                xt_sb[:, ht, qb * QW:(qb + 1) * QW],
                                start=(ht == 0),
                                stop=(ht == HT - 1 and not b_on))
                    if b_on:
                        for qb in range(QB):
                            nc.tensor.matmul(
                                p_ps[:, qb, :],
                                biasp_sb[0:1,
                                         boff + j0 * HD:boff + (j0 + 2) * HD],
                                ones_row[0:1, :], start=False, stop=True)
                    pp.append(p_ps)
                # evacuations on DVE: q gets the sigmoid gate, k is a copy
                for qb in range(QB):
                    qsl = slice(qb * QW, (qb + 1) * QW)
                    nc.vector.tensor_tensor(
                        st["qm"][:, qsl], pp[0][:, qb, :],
                        mb_all[:, pair, qsl], mult)
                    nc.vector.tensor_copy(st["km"][:, qsl],
                                          pp[1][:, qb, :])

            def emit_scores(pair, qb, ktp_range):
                """Scores + EXP for kt-pairs in ktp_range; returns expS."""
                st = pst[pair]
                qs = slice(qb * QW, (qb + 1) * QW)
                if "expS" not in st or st.get("expS_qb") != qb:
                    st["expS"] = [
                        xpool.tile([P, KT, QW], DT, tag="expS", bufs=3,
                                   name=f"ex{pair}{qb}{sub}")
                        for sub in range(2)]
                    st["expS_qb"] = qb
                expS = st["expS"]
                for kt in ktp_range:
                    s_ps = [mpool.tile([P, 2, QW], F32, tag="s2", bufs=3,
                                       name=f"sp{pair}{qb}{kt}{sub}")
                            for sub in range(2)]
                    for half in range(2):
                        for sub in range(2):
                            po = sub * 64
                            nc.tensor.matmul(
                                s_ps[sub][:, half, :],
                                st["km"][po:po + 64,
                                         (kt + half) * P:(kt + half + 1) * P],
                                st["qm"][po:po + 64, qs],
                                start=True, stop=True)
                    for sub in range(2):
                        nc.scalar.activation(expS[sub][:, kt:kt + 2, :],
                                             s_ps[sub][:], EXP, scale=0.125)

            def emit_pv_recip(pair, qb):
                """PV chains (kt-interleaved over subs) + recips on DVE."""
                st = pst[pair]
                j0 = pair * 2
                expS = st["expS"]
                c_ps = [mpool.tile([HD + 1, QW], F32, tag="pv", bufs=2,
                                   name=f"cx{pair}{qb}{sub}")
                        for sub in range(2)]
                for kt in range(KT):
                    for sub in range(2):
                        nc.tensor.matmul(c_ps[sub][:], v_sb[:, kt, j0 + sub, :],
                                         expS[sub][:, kt, :],
                                         start=(kt == 0),
                                         stop=(kt == KT - 1))
                recip16 = []
                for sub in range(2):
                    den_sb = qkpool.tile([1, QW], F32, tag="densb", bufs=2,
                                         name=f"dn{pair}{qb}{sub}")
                    nc.vector.tensor_copy(den_sb[:], c_ps[sub][64:65, :])
                    rtmp = qkpool.tile([1, QW], F32, tag="rtmp", bufs=2,
                                       name=f"rt{pair}{qb}{sub}")
                    nc.vector.reciprocal_approx_fast(rtmp[:], den_sb[:])
                    r16 = qkpool.tile([1, QW], DT, tag="recip16", bufs=2,
                                      name=f"r6{pair}{qb}{sub}")
                    nc.vector.tensor_copy(r16[:], rtmp[:])
                    recip16.append(r16)
                return {"pair": pair, "qb": qb, "c_ps": c_ps,
                        "recip16": recip16}

            def flush_rb(pend):
                """Deferred: broadcast 1/den and scale ctx into ch_sb."""
                if pend is None:
                    return
                pair, qb = pend["pair"], pend["qb"]
                qs = slice(qb * QW, (qb + 1) * QW)
                rb_ps = mpool.tile([P, QW], F32, tag="s2", bufs=3,
                                   name=f"rp{pair}{qb}")
                for sub in range(2):
                    po = sub * 64
                    nc.tensor.matmul(
                        rb_ps[po:po + 64, :], ones_row[0:1, 0:64],
                        pend["recip16"][sub][0:1, :],
                        start=True, stop=True)
                rb_sb = qkpool.tile([P, QW], DT, tag="rb", bufs=2,
                                    name=f"rs{pair}{qb}")
                nc.scalar.copy(rb_sb[:], rb_ps[:])
                for sub in range(2):
                    po = sub * 64
                    nc.vector.tensor_tensor(
                        ch_sb[po:po + 64, pair, qs],
                        pend["c_ps"][sub][0:64, :], rb_sb[po:po + 64, :], mult)

            ex_state = {}

            def emit_exchange_pre(pair):
                c = pair
                if c % 2 == 0:
                    rsum_sb = rpool.tile([P, 2, S], DT, tag="rsum", bufs=1,
                                         name=f"rsum{c}")
                    nc.sync.dma_start(
                        rsum_sb[:],
                        rs1_out[c // 2].rearrange("(o p) c -> p o c", p=P))
                    ex_state["rsum"] = rsum_sb
                nc.vector.tensor_tensor(
                    ch_sb[:, c, :], ex_state["rsum"][:, c % 2, :],
                    ch_sb[:, c, :], add)
                nc.sync.dma_start(
                    ag_in[c].rearrange("(o p) c -> p o c", p=P)[:, 0, :],
                    ch_sb[:, c, :])

            def emit_exchange_ag(pair):
                c = pair
                nc.gpsimd.collective_compute(
                    "AllGather", mybir.AluOpType.bypass,
                    replica_groups=groups,
                    ins=[ag_in[c].opt()], outs=[ag_out[c].opt()])
                # readbacks on the scalar ring so the next pair's ag_in
                # write on the sync ring is not serialized behind them
                for half in range(2):
                    nc.scalar.dma_start(
                        ctxT_sb[:, half * 4 + c, :],
                        ag_out[c, half].rearrange("(o p) c -> p o c", p=P)[:, 0, :])

            def emit_exchange_half3(h):
                # pair 3: exchange each query-half as soon as its mults land,
                # so the last AllGather is mostly off the critical path
                hs = slice(h * 512, (h + 1) * 512)
                nc.vector.tensor_tensor(
                    ch_sb[:, 3, hs], ex_state["rsum"][:, 1, hs],
                    ch_sb[:, 3, hs], add)
                nc.sync.dma_start(
                    ag3_in[h].rearrange("(o p) c -> p o c", p=P)[:, 0, :],
                    ch_sb[:, 3, hs])
                nc.gpsimd.collective_compute(
                    "AllGather", mybir.AluOpType.bypass,
                    replica_groups=groups,
                    ins=[ag3_in[h].opt()], outs=[ag3_out[h].opt()])
                for half in range(2):
                    nc.sync.dma_start(
                        ctxT_sb[:, half * 4 + 3, hs],
                        ag3_out[h, half].rearrange("(o p) c -> p o c", p=P)[:, 0, :])

            # -------- causal branch; emitted first (its EXPs overlap the ----
            # -------- main projections that follow in the PE queue) ---------
            def rs1_cb(c):
                nc.gpsimd.collective_compute(
                    "ReduceScatter", add, replica_groups=groups,
                    ins=[rs1_in[c].opt()], outs=[rs1_out[c].opt()])

            _mha256(nc, tc, mpool, mhapool, xt_sb,
                    (cwq_sb, cwk_sb, cwv_sb, cwo_sb),
                    rs1_in, ones_row, ones_col, indic, biasp_sb,
                    (CQ_OFF, CK_OFF, CV_OFF, CBO_OFF),
                    (bias_on["cq"], bias_on["ck"], bias_on["cv"],
                     bias_on["cbo"]),
                    "c", chunk_cb=rs1_cb, shared_xpool=xpool)

            # meta weights: enqueue now (slot-free sems gate the transfers,
            # but the enqueue must not sit behind the main-attention EXPs)
            mwq_sb = wpool.tile([P, HT, 512], DT, tag="wA", name="mwq")
            mwk_sb = wpool.tile([P, HT, 512], DT, tag="wB", name="mwk")
            mwv_sb = wpool.tile([P, HT, 512], DT, tag="wC", name="mwv")
            mwo_sb = wpool.tile([P, 4, H], DT, tag="wD", name="mwo")
            with tc.high_priority():
                nc.scalar.dma_start(mwq_sb[:], r3(mwq_d))
                nc.scalar.dma_start(mwk_sb[:], r3(mwk_d))
                nc.scalar.dma_start(mwv_sb[:], r3(mwv_d))
                nc.scalar.dma_start(mwo_sb[:], r3(mwo_d))

            # main attention, software-pipelined; AG per pair
            pending = None
            emit_proj_mm(0)
            for _pair in range(4):
                for _qb in range(QB):
                    emit_scores(_pair, _qb, [0, 2, 4])
                    was_qb0 = pending is not None and pending["qb"] == 0
                    flush_rb(pending)
                    pending = None
                    if _pair == 3 and was_qb0:
                        emit_exchange_half3(0)
                    emit_scores(_pair, _qb, [6])
                    pending = emit_pv_recip(_pair, _qb)
                if _pair < 3:
                    emit_proj_mm(_pair + 1)
                    flush_rb(pending)
                    pending = None
                    emit_exchange_pre(_pair)
                    emit_exchange_ag(_pair)
                else:
                    flush_rb(pending)
                    pending = None
                    emit_exchange_half3(1)

            # ===== meta branch: full in-projections over AllGathered ctx ====
            # Chains are emitted as a wavefront (3 at a time, early ctxT
            # chunks first) so the PE fills the last AllGather's latency
            # instead of head-of-line blocking on chunks 3/7.
            # meta-branch tiles reuse the causal branch's slots (now dead)
            m_qcT = mhapool.tile([P, 4, S], DT, tag="qcT", name="m_qcT")
            m_kcT = mhapool.tile([P, 4, S], DT, tag="kcT", name="m_kcT")
            m_vc = mhapool.tile([P, TT, 512], DT, tag="vc", name="m_vc")
            m_attnT = mhapool.tile([P, 4, S], DT, tag="attnT", name="m_attnT")

            # 8 q/k chains: (dst, dc) pairs, grouped 3-wide over PSUM slots
            qk_chains = [(di, dc) for dc in range(4) for di in range(2)]
            for g0 in range(0, len(qk_chains), 3):
                grp = qk_chains[g0:g0 + 3]
                tiles = {}
                for di, dc in grp:
                    tiles[(di, dc)] = mpool.tile(
                        [P, 2, QW], F32, tag="s2", bufs=3,
                        name=f"mpp{di}{dc}")
                steps = ([(hi, ht, qb) for hi, ht in enumerate(HT_ORDER[:6])
                          for qb in range(QB)]
                         + [(6 + i, HT_ORDER[6 + i], qb)
                            for qb in range(QB) for i in range(2)])
                for hi, ht, qb in steps:
                    for di, dc in grp:
                        w_sb = mwq_sb if di == 0 else mwk_sb
                        b_on = bias_on["mq"] if di == 0 else bias_on["mk"]
                        nc.tensor.matmul(
                            tiles[(di, dc)][:, qb, :],
                            w_sb[:, ht, dc * P:(dc + 1) * P],
                            ctxT_sb[:, ht, qb * QW:(qb + 1) * QW],
                            start=(hi == 0),
                            stop=(hi == HT - 1 and not b_on))
                for di, dc in grp:
                    b_on = bias_on["mq"] if di == 0 else bias_on["mk"]
                    boff = MQ_OFF if di == 0 else MK_OFF
                    if b_on:
                        for qb in range(QB):
                            nc.tensor.matmul(
                                tiles[(di, dc)][:, qb, :],
                                biasp_sb[0:1, boff + dc * P:boff + (dc + 1) * P],
                                ones_row[0:1, :], start=False, stop=True)
                    dst = m_qcT if di == 0 else m_kcT
                    nc.scalar.copy(
                        dst[:, dc, :].rearrange("p (a b) -> p a b", a=2),
                        tiles[(di, dc)])
            # v chains, 2-wide
            for t0 in range(0, TT, 2):
                vt = {}
                for tt in range(t0, t0 + 2):
                    vt[tt] = mpool.tile([P, QW], F32, tag="pv", bufs=2,
                                        name=f"mv{tt}")
                for hi, ht in enumerate(HT_ORDER):
                    for tt in range(t0, t0 + 2):
                        nc.tensor.matmul(
                            vt[tt][:],
                            ctxT_sb[:, ht, tt * P:(tt + 1) * P],
                            mwv_sb[:, ht, :],
                            start=(hi == 0),
                            stop=(hi == HT - 1 and not bias_on["mv"]))
                for tt in range(t0, t0 + 2):
                    if bias_on["mv"]:
                        nc.tensor.matmul(vt[tt][:], ones_row[0:1, 0:P],
                                         biasp_sb[0:1, MV_OFF:MV_OFF + 512],
                                         start=False, stop=True)
                    nc.scalar.copy(m_vc[:, tt, :], vt[tt][:])

            # final-Wo weights: enqueue ahead of the meta-attention EXPs
            wo_a = wpool.tile([P, 4, H], DT, tag="wA", name="wo_a")
            wo_b = wpool.tile([P, 4, H], DT, tag="wB", name="wo_b")
            with tc.high_priority():
                nc.scalar.dma_start(wo_a[:], r3(wo_d)[:, 0:4, :])
                nc.scalar.dma_start(wo_b[:], r3(wo_d)[:, 4:8, :])

            # meta attention + out-projection (kept local; summed on host)
            mp_sb = apool.tile([P, HT, S], DT, tag="xt_mp")
            _attn256(nc, tc, mpool, (m_qcT, m_kcT, m_vc, m_attnT), mwo_sb,
                     mp_sb, ones_row, ones_col, biasp_sb,
                     MBO_OFF, bias_on["mbo"], "c", out_sb=True,
                     xpool=xpool)

        # ===== Z = 0.425*ctx + mp_local ; out_partial = Z^T.T @ Wo (full) ====
        # wo reuses the (dead) meta in-proj slots; final matmuls consume the
        # blended chunks in meta out-proj emission order (PT_ORDER).
        with tc.tile_pool(name="fin", bufs=1) as fpool, \
             tc.tile_pool(name="fstage", bufs=3) as spool:
            zs_sb = fpool.tile([P, HT, S], DT)    # final blended Z^T
            for pt in PT_ORDER:
                nc.vector.scalar_tensor_tensor(zs_sb[:, pt, :],
                                               ctxT_sb[:, pt, :], 0.425,
                                               mp_sb[:, pt, :], mult, add)
            for tt in range(TT):
                o_ps = mpool.tile([P, 2, QW], F32, tag="s2", bufs=3)
                for cb in range(2):
                    for ci, ct in enumerate(PT_ORDER):
                        wo_sb = wo_a if ct < 4 else wo_b
                        nc.tensor.matmul(o_ps[:, cb, :],
                                         zs_sb[:, ct, tt * P:(tt + 1) * P],
                                         wo_sb[:, ct % 4, cb * 512:(cb + 1) * 512],
                                         start=(ci == 0), stop=(ci == HT - 1))
                o_sb = spool.tile([P, 2, QW], DT, tag="outst")
                if tt % 2 == 0:
                    nc.vector.tensor_copy(o_sb[:], o_ps[:])
                else:
                    nc.scalar.copy(o_sb[:], o_ps[:])
                nc.sync.dma_start(out_v[:, tt, :],
                                  o_sb[:, :, :].rearrange("p a b -> p (a b)"))


def _mha256(nc, tc, mpool, mhapool, x_sb, w_tiles, out_dram,
            ones_row, ones_col, indic, biasp_sb, boffs, bflags, prefix,
            chunk_cb=None, shared_xpool=None):
    """Causal 256-dim-head MHA branch: full in-proj + attention + out-proj.

    x_sb     [P, HT, S]  input ^T
    w_tiles  (wq, wk, wv, wo) SBUF tiles, pre-DMA'd by the caller:
             wq/k/v [P, HT, 512] in-proj slices (my 2 heads),
             wo [P, 4, H] out-proj rows slice (pre-scaled by blend weight)
    out_dram [2, 2, 256, H] dram bounce for the chunked ReduceScatter
    chunk_cb(c) is invoked right after chunk c's out-proj tiles are emitted
    """
    qoff, koff, voff, booff = boffs
    bq_on, bk_on, bv_on, bo_on = bflags

    wq_sb, wk_sb, wv_sb, wo_sb = w_tiles

    import contextlib
    _st = contextlib.ExitStack()
    qcT = mhapool.tile([P, 4, S], DT, tag="qcT", name=f"{prefix}_qcT")
    kcT = mhapool.tile([P, 4, S], DT, tag="kcT", name=f"{prefix}_kcT")
    vc = mhapool.tile([P, TT, 512], DT, tag="vc", name=f"{prefix}_vc")
    attnT = mhapool.tile([P, 4, S], DT, tag="attnT", name=f"{prefix}_attnT")
    if shared_xpool is None:
        xpool = _st.enter_context(tc.tile_pool(name=f"{prefix}exp", bufs=2))
    else:
        xpool = shared_xpool

    # in-projections q^T, k^T  (4 chunks of 128 rows = 2 heads x 2)
    for dc in range(4):
        for dst, w_sb, boff, b_on in ((qcT, wq_sb, qoff, bq_on),
                                      (kcT, wk_sb, koff, bk_on)):
            p_ps = mpool.tile([P, 2, QW], F32, tag="s2", bufs=3)
            for ht in range(HT):
                for qb in range(QB):
                    nc.tensor.matmul(p_ps[:, qb, :],
                                     w_sb[:, ht, dc * P:(dc + 1) * P],
                                     x_sb[:, ht, qb * QW:(qb + 1) * QW],
                                     start=(ht == 0),
                                     stop=(ht == HT - 1 and not b_on))
            if b_on:
                for qb in range(QB):
                    nc.tensor.matmul(
                        p_ps[:, qb, :],
                        biasp_sb[0:1, boff + dc * P:boff + (dc + 1) * P],
                        ones_row[0:1, :], start=False, stop=True)
            nc.vector.tensor_copy(
                dst[:, dc, :].rearrange("p (a b) -> p a b", a=2),
                p_ps[:])
    # v (normal layout)
    for tt in range(TT):
        v_ps = mpool.tile([P, QW], F32, tag="pv", bufs=2)
        for ht in range(HT):
            nc.tensor.matmul(v_ps[:], x_sb[:, ht, tt * P:(tt + 1) * P],
                             wv_sb[:, ht, :],
                             start=(ht == 0),
                             stop=(ht == HT - 1 and not bv_on))
        if bv_on:
            nc.tensor.matmul(v_ps[:], ones_row[0:1, 0:P],
                             biasp_sb[0:1, voff:voff + 512],
                             start=False, stop=True)
        nc.vector.tensor_copy(vc[:, tt, :], v_ps[:])

    _attn256(nc, tc, mpool, (qcT, kcT, vc, attnT), wo_sb, out_dram,
             ones_row, ones_col, biasp_sb, booff, bo_on, prefix,
             chunk_cb=chunk_cb, out_sb=False, xpool=xpool)
    _st.close()


def _attn256(nc, tc, mpool, t_tiles, wo_sb, out_dram,
             ones_row, ones_col, biasp_sb, booff, bo_on, prefix,
             chunk_cb=None, out_sb=False, xpool=None):
    """Attention + out-projection for the 256-dim-head branches.

    Per block: scores -> EXP -> den matmuls -> PV chains (the den->recip
    roundtrip runs on DVE underneath the PV chains) -> rb broadcast ->
    normalizing mults (DVE)."""
    mult = mybir.AluOpType.mult
    EXP = mybir.ActivationFunctionType.Exp
    qcT, kcT, vc, attnT = t_tiles

    # attention per head
    for jc in range(CH_LOC):
        for qb in range(QB):
            qs = slice(qb * QW, (qb + 1) * QW)
            expS = xpool.tile([P, KT, QW], DT, tag="expS", bufs=3)
            for kt in range(0, KT, 2):
                s_ps = mpool.tile([P, 2, QW], F32, tag="s2", bufs=3)
                for half in range(2):
                    for dc in range(2):
                        nc.tensor.matmul(
                            s_ps[:, half, :],
                            kcT[:, jc * 2 + dc, (kt + half) * P:(kt + half + 1) * P],
                            qcT[:, jc * 2 + dc, qs],
                            start=(dc == 0), stop=(dc == 1))
                nc.scalar.activation(expS[:, kt:kt + 2, :], s_ps[:], EXP,
                                     scale=0.0625)
            den_ps = mpool.tile([1, QW], F32, tag="pv", bufs=2)
            for kt in range(KT):
                nc.tensor.matmul(den_ps[:], ones_col[:, 0:1], expS[:, kt, :],
                                 start=(kt == 0), stop=(kt == KT - 1))
            # recip path on DVE, overlapped with the PV chains below
            den_row = xpool.tile([1, QW], F32, tag=f"{prefix}denrow", bufs=1)
            nc.vector.tensor_copy(den_row[:], den_ps[:])
            recip = xpool.tile([1, QW], F32, tag=f"{prefix}recip", bufs=1)
            nc.vector.reciprocal_approx_fast(recip[:], den_row[:])
            recip16 = xpool.tile([1, QW], DT, tag=f"{prefix}recip16", bufs=1)
            nc.vector.tensor_copy(recip16[:], recip[:])
            pv_ps = []
            for dc in range(2):
                c_ps = mpool.tile([P, QW], F32, tag="pv", bufs=2)
                for kt in range(KT):
                    nc.tensor.matmul(c_ps[:],
                                     vc[:, kt, (jc * 2 + dc) * P:(jc * 2 + dc + 1) * P],
                                     expS[:, kt, :],
                                     start=(kt == 0), stop=(kt == KT - 1))
                pv_ps.append(c_ps)
            rb_ps = mpool.tile([P, QW], F32, tag="s2", bufs=3)
            nc.tensor.matmul(rb_ps[:], ones_row[0:1, 0:P], recip16[:],
                             start=True, stop=True)
            rb_sb = xpool.tile([P, QW], DT, tag=f"{prefix}rb", bufs=1)
            nc.vector.tensor_copy(rb_sb[:], rb_ps[:])
            for dc in range(2):
                nc.vector.tensor_tensor(attnT[:, jc * 2 + dc, qs],
                                        pv_ps[dc][:], rb_sb[:], mult)

    # out-projection: [512 local dims] x [H out rows], chunk-pipelined order,
    # staged straight to the collective's dram bounce buffer
    for pi, pt in enumerate(PT_ORDER):
        c, half, r = (pt % 4) // 2, pt // 4, pt % 2
        op_ps = mpool.tile([P, 2, QW], F32, tag="s2", bufs=3)
        for qb in range(QB):
            for ct in range(4):
                nc.tensor.matmul(op_ps[:, qb, :],
                                 wo_sb[:, ct, pt * P:(pt + 1) * P],
                                 attnT[:, ct, qb * QW:(qb + 1) * QW],
                                 start=(ct == 0),
                                 stop=(ct == 3 and not bo_on))
            if bo_on:
                nc.tensor.matmul(
                    op_ps[:, qb, :],
                    biasp_sb[0:1, booff + pt * P:booff + (pt + 1) * P],
                    ones_row[0:1, :], start=False, stop=True)
        if out_sb:
            nc.scalar.copy(
                out_dram[:, pt, :].rearrange("p (a b) -> p a b", a=2), op_ps[:])
        else:
            ost = xpool.tile([P, 2, QW], DT, tag=f"{prefix}ost", bufs=2)
            nc.vector.tensor_copy(ost[:], op_ps[:])
            nc.sync.dma_start(
                out_dram[c, half].rearrange("(o p) c -> p o c", p=P)[:, r, :],
                ost[:, :, :].rearrange("p a b -> p (a b)"))
        if chunk_cb is not None and pi == 3:
            chunk_cb(0)
    if chunk_cb is not None:
        chunk_cb(1)


_CACHE = {}


def _get_compiled(bias_key):
    if bias_key in _CACHE:
        return _CACHE[bias_key]
    bias_on = dict(bias_key)
    nc = bacc.Bacc("TRN2", target_bir_lowering=False, debug=False,
                   num_devices=N_CORES)
    with tile.TileContext(nc) as tc:
        _emit(nc, tc, bias_on)
    nc.compile()
    _CACHE[bias_key] = nc
    return nc


def _bias_key(inp):
    bq, bk, bv = inp["bq"], inp["bk"], inp["bv"]
    ca_bin, ca_bout = inp["ca_bin"], inp["ca_bout"]
    mc_bin, mc_bout = inp["mc_bin"], inp["mc_bout"]
    bias_on = {
        "bq": bool(np.any(bq)), "bk": bool(np.any(bk)), "bv": bool(np.any(bv)),
        "cq": bool(np.any(ca_bin[:H])), "ck": bool(np.any(ca_bin[H:2 * H])),
        "cv": bool(np.any(ca_bin[2 * H:])), "cbo": bool(np.any(ca_bout)),
        "mq": bool(np.any(mc_bin[:H])), "mk": bool(np.any(mc_bin[H:2 * H])),
        "mv": bool(np.any(mc_bin[2 * H:])), "mbo": bool(np.any(mc_bout)),
    }
    return tuple(sorted(bias_on.items()))


def _shard_in_maps(inp):
    CAUSAL_W = 0.7
    META_W = ((0.9 - 0.8) / 0.2) * 0.3
    hidden_states = inp["hidden_states"]
    cons_vec, am_W, am_b = inp["cons_vec"], inp["am_W"], inp["am_b"]
    cg_W, cg_b = inp["cg_W"], inp["cg_b"]
    Wq, bq, Wk, bk, Wv, bv = (inp["Wq"], inp["bq"], inp["Wk"], inp["bk"],
                              inp["Wv"], inp["bv"])
    ca_Win, ca_bin, ca_Wout, ca_bout = (inp["ca_Win"], inp["ca_bin"],
                                        inp["ca_Wout"], inp["ca_bout"])
    mc_Win, mc_bin, mc_Wout, mc_bout = (inp["mc_Win"], inp["mc_bin"],
                                        inp["mc_Wout"], inp["mc_bout"])
    Wo = inp["Wo"]

    hs = np.asarray(hidden_states, np.float32)
    am_vec = np.asarray(cons_vec, np.float32) @ np.asarray(am_W, np.float32) \
        + np.asarray(am_b, np.float32)
    modb_full = np.asarray(cg_b, np.float32) + am_vec          # [16]

    def b16(a):
        return np.ascontiguousarray(np.asarray(a, np.float32)).astype(BF16)

    in_maps = []
    for c in range(N_CORES):
        b, h = c // 2, c % 2
        cols = slice(h * 512, (h + 1) * 512)
        rows_own = slice(h * 512, (h + 1) * 512)
        biasp = np.zeros(NBIAS, np.float32)
        biasp[BQ_OFF:BQ_OFF + 512] = np.asarray(bq, np.float32)[cols]
        biasp[BK_OFF:BK_OFF + 512] = np.asarray(bk, np.float32)[cols]
        biasp[BV_OFF:BV_OFF + 512] = 0.3 * np.asarray(bv, np.float32)[cols]
        biasp[CQ_OFF:CQ_OFF + 512] = np.asarray(ca_bin, np.float32)[0:H][cols]
        biasp[CK_OFF:CK_OFF + 512] = np.asarray(ca_bin, np.float32)[H:2 * H][cols]
        biasp[CV_OFF:CV_OFF + 512] = np.asarray(ca_bin, np.float32)[2 * H:][cols]
        biasp[MQ_OFF:MQ_OFF + 512] = np.asarray(mc_bin, np.float32)[0:H][cols]
        biasp[MK_OFF:MK_OFF + 512] = np.asarray(mc_bin, np.float32)[H:2 * H][cols]
        biasp[MV_OFF:MV_OFF + 512] = np.asarray(mc_bin, np.float32)[2 * H:][cols]
        if h == 0:
            biasp[CBO_OFF:CBO_OFF + H] = CAUSAL_W * np.asarray(ca_bout, np.float32)
            biasp[MBO_OFF:MBO_OFF + H] = META_W * np.asarray(mc_bout, np.float32)
        m = {
            "xt": b16(hs[b].T),
            "wq": b16(Wq[:, cols]),
            "wk": b16(Wk[:, cols]),
            "wv": b16(0.3 * np.asarray(Wv, np.float32)[:, cols]),
            "cgw": b16(np.asarray(cg_W, np.float32)[:, h * 8:(h + 1) * 8]),
            "modb": np.ascontiguousarray(
                modb_full[h * 8:(h + 1) * 8].reshape(8, 1)),
            "cwq": b16(np.asarray(ca_Win, np.float32)[:, 0:H][:, cols]),
            "cwk": b16(np.asarray(ca_Win, np.float32)[:, H:2 * H][:, cols]),
            "cwv": b16(np.asarray(ca_Win, np.float32)[:, 2 * H:][:, cols]),
            "cwo": b16(CAUSAL_W * np.asarray(ca_Wout, np.float32)[cols, :]),
            "mwq": b16(np.asarray(mc_Win, np.float32)[:, 0:H][:, cols]),
            "mwk": b16(np.asarray(mc_Win, np.float32)[:, H:2 * H][:, cols]),
            "mwv": b16(np.asarray(mc_Win, np.float32)[:, 2 * H:][:, cols]),
            "mwo": b16(META_W * np.asarray(mc_Wout, np.float32)[cols, :]),
            "wo": b16(np.asarray(Wo, np.float32)),
            "biasp": biasp.reshape(1, NBIAS).astype(BF16),
        }
        in_maps.append(m)
    return in_maps


def kernel(**inputs):
    _install_ntff_hook()
    nc = _get_compiled(_bias_key(inputs))
    in_maps = _shard_in_maps(inputs)
    res = None
    for attempt in range(3):
        try:
            res = bass_utils.run_bass_kernel_spmd(nc, in_maps,
                                                  core_ids=list(range(N_CORES)))
            break
        except Exception:
            if attempt == 2:
                raise
            import time as _time
            _time.sleep(2.0)
    out = np.zeros((4, S, H), np.float32)
    bo_f = np.asarray(inputs["bo"], np.float32)
    for b in range(4):
        out[b] = (np.asarray(res.results[2 * b]["out"], np.float32)
                  + np.asarray(res.results[2 * b + 1]["out"], np.float32)
                  + bo_f)
    return out
